# revision 5
# baseline (speedup 1.0000x reference)
"""Depthwise causal FIR conv1d (B=4, L=4096, H=16, D=64, K=7) on 8 trn2 cores.

Sharding: channels C = H*D = 1024 split across 8 cores -> 128 channels/core
(= SBUF partition count). Channels live on partitions, time on the free axis.

Per-core compute split:
  - tap K-1 (the ~1.0 "delta" tap) in full fp32 on ScalarE (Copy activation
    with per-partition scale),
  - taps 0..K-2 (small corrections) as diagonal matmuls in bf16 on
    TensorE, accumulated across taps in fp32 PSUM (tap shift =
    moving-operand column offset),
  - VectorE casts x to bf16 and adds PSUM corrections into the main-tap
    result per 512-col chunk.
"""

import numpy as np

B, L, H, D, K = 4, 4096, 16, 64, 7
C = H * D          # 1024 channels
NCORES = 8
CPC = C // NCORES  # 128 channels per core
LP = L + K - 1     # padded row length: 4102
NCHUNK = 512       # psum-bank-sized output chunk
NCH = L // NCHUNK  # chunks per batch row


def _make_tile_context_cls():
    import concourse.mybir as mybir
    from concourse.tile import TileContext
    from concourse.vector_clock import ScopedClock

    class PatchedTileContext(TileContext):
        """The walrus build in this container only accepts ONE sync-wait
        command per instruction.  Tile routinely attaches several (slot
        release + cross-engine RAW deps).  Split every multi-wait
        instruction into same-engine single-wait NoOp prefixes, and do the
        same for the kernel-tail drain."""

        def _split_waits(self, inst, emit):
            si = inst.sync_info
            if si is None or not si.on_wait or len(si.on_wait) <= 1:
                return
            waits = list(si.on_wait)
            inst.sync_info = mybir.SyncInfo(
                on_wait=waits[:1], on_update=list(si.on_update)
            )
            for k, w in enumerate(waits[1:]):
                nop = mybir.InstNoOp(
                    name=f"{inst.name}_xw{k}", ins=[], outs=[], engine=inst.engine
                )
                nop.sync_info = mybir.SyncInfo(on_wait=[w], on_update=[])
                emit(nop)

        def _lower_ordered_insts(self, ordered):
            patched = {}
            for bb_name, insts in ordered.items():
                out = []
                for inst in insts:
                    self._split_waits(inst, out.append)
                    out.append(inst)
                patched[bb_name] = out
            return super()._lower_ordered_insts(patched)

        def _drain_and_barrier(self, tick_clock, wait_clock):
            carrier = self.nc.sync.nop(nofuse=True)
            wait_clock.add_sem_waits(
                carrier.ins, ScopedClock({None: tick_clock.global_clock})
            )
            si = carrier.ins.sync_info
            if si is not None and si.on_wait and len(si.on_wait) > 1:
                waits = list(si.on_wait)
                carrier.ins.sync_info = mybir.SyncInfo(
                    on_wait=waits[:1], on_update=list(si.on_update)
                )
                for w in waits[1:]:
                    nop = self.nc.sync.nop(nofuse=True)
                    nop.ins.sync_info = mybir.SyncInfo(on_wait=[w], on_update=[])
            self.nc.sync.drain()
            self.nc.all_engine_barrier()
            popped = self.nc._tile_sem_poison_stack.pop()
            assert popped is self._sem_poison
            self.nc.clear_and_free_semaphores(list(self.sems.allocated().values()))
            self.nc.all_engine_barrier()

    return PatchedTileContext


def _build_program():
    import concourse.bass as bass
    import concourse.mybir as mybir

    TileContext = _make_tile_context_cls()

    f32 = mybir.dt.float32
    bf16 = mybir.dt.bfloat16

    nc = bass.Bass()
    x_d = nc.declare_dram_parameter("x", [B, CPC, LP], f32, isOutput=False)
    wd_d = nc.declare_dram_parameter("wd", [CPC, (K - 1) * CPC], bf16, isOutput=False)
    w6_d = nc.declare_dram_parameter("w6", [CPC, 1], f32, isOutput=False)
    y_d = nc.declare_dram_parameter("y", [B, CPC, L], f32, isOutput=True)

    with TileContext(nc) as tc:
        with (
            tc.tile_pool(name="w", bufs=1) as wpool,
            tc.tile_pool(name="xb", bufs=3) as xpool,
            tc.tile_pool(name="xh", bufs=3) as xhpool,
            tc.tile_pool(name="yb", bufs=3) as ypool,
            tc.tile_pool(name="ps", bufs=8, space="PSUM") as pspool,
        ):
            wtile = wpool.tile([CPC, (K - 1) * CPC], bf16)
            nc.sync.dma_start(out=wtile[:], in_=wd_d[:])
            w6tile = wpool.tile([CPC, 1], f32)
            nc.sync.dma_start(out=w6tile[:], in_=w6_d[:])

            for b in range(B):
                xt = xpool.tile([CPC, LP], f32)
                nc.sync.dma_start(out=xt[:], in_=x_d[b])
                # bf16 copy of the padded row for the TensorE correction taps
                xh = xhpool.tile([CPC, LP], bf16)
                nc.vector.tensor_copy(out=xh[:], in_=xt[:])
                yt = ypool.tile([CPC, L], f32)
                # main tap: yt[c, l] = x[c, l] * w[c, K-1]
                nc.scalar.activation(
                    yt[:],
                    xt[:, K - 1 : LP],
                    mybir.ActivationFunctionType.Copy,
                    bias=0.0,
                    scale=w6tile[:, 0:1],
                )
                for j in range(NCH):
                    ps = pspool.tile([CPC, NCHUNK], f32)
                    for k in range(K - 1):
                        nc.tensor.matmul(
                            ps[:],
                            lhsT=wtile[:, k * CPC : (k + 1) * CPC],
                            rhs=xh[:, j * NCHUNK + k : j * NCHUNK + k + NCHUNK],
                            start=(k == 0),
                            stop=(k == K - 2),
                        )
                    nc.vector.tensor_add(
                        out=yt[:, j * NCHUNK : (j + 1) * NCHUNK],
                        in0=yt[:, j * NCHUNK : (j + 1) * NCHUNK],
                        in1=ps[:],
                    )
                nc.sync.dma_start(out=y_d[b], in_=yt[:])
    return nc


def _pack_inputs(x: np.ndarray, filters: np.ndarray):
    import ml_dtypes

    xr = np.asarray(x, dtype=np.float32).reshape(B, L, C)
    w = np.asarray(filters, dtype=np.float32).reshape(C, K)

    xpad = np.zeros((B, C, LP), dtype=np.float32)
    xpad[:, :, K - 1 :] = xr.transpose(0, 2, 1)

    in_maps = []
    for i in range(NCORES):
        c0 = i * CPC
        xs = np.ascontiguousarray(xpad[:, c0 : c0 + CPC, :])
        wc = w[c0 : c0 + CPC]
        wd = np.zeros((CPC, (K - 1) * CPC), dtype=ml_dtypes.bfloat16)
        idx = np.arange(CPC)
        for k in range(K - 1):
            wd[idx, k * CPC + idx] = wc[:, k].astype(ml_dtypes.bfloat16)
        w6 = np.ascontiguousarray(wc[:, K - 1 : K])
        in_maps.append({"x": xs, "wd": wd, "w6": w6})
    return in_maps


def _unpack_outputs(results):
    y = np.empty((B, C, L), dtype=np.float32)
    for i in range(NCORES):
        y[:, i * CPC : (i + 1) * CPC, :] = results[i]["y"]
    return np.ascontiguousarray(y.transpose(0, 2, 1)).reshape(B, L, H, D)


def _run(x: np.ndarray, filters: np.ndarray, trace: bool = False):
    from concourse.bass_utils import run_bass_kernel_spmd

    nc = _build_program()
    in_maps = _pack_inputs(x, filters)
    res = run_bass_kernel_spmd(nc, in_maps, list(range(NCORES)), trace=trace)
    return _unpack_outputs(res.results), res


def kernel(x: np.ndarray, filters: np.ndarray) -> np.ndarray:
    out, _ = _run(x, filters, trace=False)
    return out


# revision 7
# speedup vs baseline: 58.6307x; 58.6307x over previous
"""Depthwise causal FIR conv1d (B=4, L=4096, H=16, D=64, K=7) on 8 trn2 cores.

Sharding: channels C = H*D = 1024 split across 8 cores -> 128 channels/core
(= SBUF partition count). Channels live on partitions, time on the free axis.

Per-core compute split:
  - tap K-1 (the ~1.0 "delta" tap) in full fp32 on ScalarE (Copy activation
    with per-partition scale),
  - taps 0..K-2 (small corrections) as diagonal matmuls in bf16 on
    TensorE, accumulated across taps in fp32 PSUM (tap shift =
    moving-operand column offset),
  - VectorE casts x to bf16 and adds PSUM corrections into the main-tap
    result per 512-col chunk.
"""

import numpy as np

B, L, H, D, K = 4, 4096, 16, 64, 7
C = H * D          # 1024 channels
NCORES = 8
CPC = C // NCORES  # 128 channels per core
LP = L + K - 1     # padded row length: 4102
NCHUNK = 512       # psum-bank-sized output chunk
NCH = L // NCHUNK  # chunks per batch row


def _make_tile_context_cls():
    import concourse.mybir as mybir
    from concourse.tile import TileContext
    from concourse.vector_clock import ScopedClock

    class PatchedTileContext(TileContext):
        """The walrus build in this container only accepts ONE sync-wait
        command per instruction.  Tile routinely attaches several (slot
        release + cross-engine RAW deps).  Split every multi-wait
        instruction into same-engine single-wait NoOp prefixes, and do the
        same for the kernel-tail drain."""

        def _split_waits(self, inst, emit):
            si = inst.sync_info
            if si is None or not si.on_wait or len(si.on_wait) <= 1:
                return
            waits = list(si.on_wait)
            inst.sync_info = mybir.SyncInfo(
                on_wait=waits[:1], on_update=list(si.on_update)
            )
            for k, w in enumerate(waits[1:]):
                nop = mybir.InstNoOp(
                    name=f"{inst.name}_xw{k}", ins=[], outs=[], engine=inst.engine
                )
                nop.sync_info = mybir.SyncInfo(on_wait=[w], on_update=[])
                emit(nop)

        def _lower_ordered_insts(self, ordered):
            patched = {}
            for bb_name, insts in ordered.items():
                out = []
                for inst in insts:
                    self._split_waits(inst, out.append)
                    out.append(inst)
                patched[bb_name] = out
            return super()._lower_ordered_insts(patched)

        def _drain_and_barrier(self, tick_clock, wait_clock):
            carrier = self.nc.sync.nop(nofuse=True)
            wait_clock.add_sem_waits(
                carrier.ins, ScopedClock({None: tick_clock.global_clock})
            )
            si = carrier.ins.sync_info
            if si is not None and si.on_wait and len(si.on_wait) > 1:
                waits = list(si.on_wait)
                carrier.ins.sync_info = mybir.SyncInfo(
                    on_wait=waits[:1], on_update=list(si.on_update)
                )
                for w in waits[1:]:
                    nop = self.nc.sync.nop(nofuse=True)
                    nop.ins.sync_info = mybir.SyncInfo(on_wait=[w], on_update=[])
            self.nc.sync.drain()
            self.nc.all_engine_barrier()
            popped = self.nc._tile_sem_poison_stack.pop()
            assert popped is self._sem_poison
            self.nc.clear_and_free_semaphores(list(self.sems.allocated().values()))
            self.nc.all_engine_barrier()

    return PatchedTileContext


def _build_program(repeat: int = 1):
    import concourse.bass as bass
    import concourse.mybir as mybir

    TileContext = _make_tile_context_cls()

    f32 = mybir.dt.float32
    bf16 = mybir.dt.bfloat16

    nc = bass.Bass()
    x_d = nc.declare_dram_parameter("x", [B, CPC, LP], f32, isOutput=False)
    wd_d = nc.declare_dram_parameter("wd", [CPC, (K - 1) * CPC], bf16, isOutput=False)
    w6_d = nc.declare_dram_parameter("w6", [CPC, 1], f32, isOutput=False)
    y_d = nc.declare_dram_parameter("y", [B, CPC, L], f32, isOutput=True)

    with TileContext(nc) as tc:
        with (
            tc.tile_pool(name="w", bufs=1) as wpool,
            tc.tile_pool(name="xb", bufs=3) as xpool,
            tc.tile_pool(name="xh", bufs=3) as xhpool,
            tc.tile_pool(name="yb", bufs=3) as ypool,
            tc.tile_pool(name="ps", bufs=8, space="PSUM") as pspool,
        ):
            wtile = wpool.tile([CPC, (K - 1) * CPC], bf16)
            nc.sync.dma_start(out=wtile[:], in_=wd_d[:])
            w6tile = wpool.tile([CPC, 1], f32)
            nc.sync.dma_start(out=w6tile[:], in_=w6_d[:])

            for b in [b for _ in range(repeat) for b in range(B)]:
                xt = xpool.tile([CPC, LP], f32)
                nc.sync.dma_start(out=xt[:], in_=x_d[b])
                # bf16 copy of the padded row for the TensorE correction taps
                xh = xhpool.tile([CPC, LP], bf16)
                nc.vector.tensor_copy(out=xh[:], in_=xt[:])
                yt = ypool.tile([CPC, L], f32)
                # main tap: yt[c, l] = x[c, l] * w[c, K-1]
                nc.scalar.activation(
                    yt[:],
                    xt[:, K - 1 : LP],
                    mybir.ActivationFunctionType.Copy,
                    bias=0.0,
                    scale=w6tile[:, 0:1],
                )
                for j in range(NCH):
                    ps = pspool.tile([CPC, NCHUNK], f32)
                    for k in range(K - 1):
                        nc.tensor.matmul(
                            ps[:],
                            lhsT=wtile[:, k * CPC : (k + 1) * CPC],
                            rhs=xh[:, j * NCHUNK + k : j * NCHUNK + k + NCHUNK],
                            start=(k == 0),
                            stop=(k == K - 2),
                        )
                    nc.vector.tensor_add(
                        out=yt[:, j * NCHUNK : (j + 1) * NCHUNK],
                        in0=yt[:, j * NCHUNK : (j + 1) * NCHUNK],
                        in1=ps[:],
                    )
                nc.sync.dma_start(out=y_d[b], in_=yt[:])
    return nc


def _pack_inputs(x: np.ndarray, filters: np.ndarray):
    import ml_dtypes

    xr = np.asarray(x, dtype=np.float32).reshape(B, L, C)
    w = np.asarray(filters, dtype=np.float32).reshape(C, K)

    xpad = np.zeros((B, C, LP), dtype=np.float32)
    xpad[:, :, K - 1 :] = xr.transpose(0, 2, 1)

    in_maps = []
    for i in range(NCORES):
        c0 = i * CPC
        xs = np.ascontiguousarray(xpad[:, c0 : c0 + CPC, :])
        wc = w[c0 : c0 + CPC]
        wd = np.zeros((CPC, (K - 1) * CPC), dtype=ml_dtypes.bfloat16)
        idx = np.arange(CPC)
        for k in range(K - 1):
            wd[idx, k * CPC + idx] = wc[:, k].astype(ml_dtypes.bfloat16)
        w6 = np.ascontiguousarray(wc[:, K - 1 : K])
        in_maps.append({"x": xs, "wd": wd, "w6": w6})
    return in_maps


def _unpack_outputs(results):
    y = np.empty((B, C, L), dtype=np.float32)
    for i in range(NCORES):
        y[:, i * CPC : (i + 1) * CPC, :] = results[i]["y"]
    return np.ascontiguousarray(y.transpose(0, 2, 1)).reshape(B, L, H, D)


def _run(x: np.ndarray, filters: np.ndarray, trace: bool = False):
    from concourse.bass_utils import run_bass_kernel_spmd

    nc = _build_program()
    in_maps = _pack_inputs(x, filters)
    res = run_bass_kernel_spmd(nc, in_maps, list(range(NCORES)), trace=trace)
    return _unpack_outputs(res.results), res


def kernel(x: np.ndarray, filters: np.ndarray) -> np.ndarray:
    out, _ = _run(x, filters, trace=False)
    return out


# revision 23
# speedup vs baseline: 80.3708x; 1.3708x over previous
"""Depthwise causal FIR conv1d (B=4, L=4096, H=16, D=64, K=7) on 8 trn2 cores.

Sharding: channels C = H*D = 1024 split across 8 cores -> 128 channels/core
(= SBUF partition count). Channels live on partitions, time on the free axis.

Per-core compute split:
  - tap K-1 (the ~1.0 "delta" tap) in full fp32 on ScalarE (Copy activation
    with per-partition scale),
  - taps 0..K-2 (small corrections) as diagonal matmuls in bf16 on
    TensorE, accumulated across taps in fp32 PSUM (tap shift =
    moving-operand column offset),
  - VectorE casts x to bf16 and adds PSUM corrections into the main-tap
    result per 512-col chunk.
"""

import numpy as np

B, L, H, D, K = 4, 4096, 16, 64, 7
C = H * D          # 1024 channels
NCORES = 8
CPC = C // NCORES  # 128 channels per core
LP = L + K - 1     # padded row length: 4102
NCHUNK = 512       # psum-bank-sized output chunk
NCH = L // NCHUNK  # chunks per batch row


def _make_tile_context_cls():
    import concourse.mybir as mybir
    from concourse.tile import TileContext
    from concourse.vector_clock import ScopedClock

    class PatchedTileContext(TileContext):
        """The walrus build in this container only accepts ONE sync-wait
        command per instruction.  Tile routinely attaches several (slot
        release + cross-engine RAW deps).  Split every multi-wait
        instruction into same-engine single-wait NoOp prefixes, and do the
        same for the kernel-tail drain."""

        def _split_waits(self, inst, emit):
            si = inst.sync_info
            if si is None or not si.on_wait or len(si.on_wait) <= 1:
                return
            waits = list(si.on_wait)
            inst.sync_info = mybir.SyncInfo(
                on_wait=waits[:1], on_update=list(si.on_update)
            )
            for k, w in enumerate(waits[1:]):
                nop = mybir.InstNoOp(
                    name=f"{inst.name}_xw{k}", ins=[], outs=[], engine=inst.engine
                )
                nop.sync_info = mybir.SyncInfo(on_wait=[w], on_update=[])
                emit(nop)

        def _lower_ordered_insts(self, ordered):
            patched = {}
            for bb_name, insts in ordered.items():
                out = []
                for inst in insts:
                    self._split_waits(inst, out.append)
                    out.append(inst)
                patched[bb_name] = out
            return super()._lower_ordered_insts(patched)

        def _drain_and_barrier(self, tick_clock, wait_clock):
            carrier = self.nc.sync.nop(nofuse=True)
            wait_clock.add_sem_waits(
                carrier.ins, ScopedClock({None: tick_clock.global_clock})
            )
            si = carrier.ins.sync_info
            if si is not None and si.on_wait and len(si.on_wait) > 1:
                waits = list(si.on_wait)
                carrier.ins.sync_info = mybir.SyncInfo(
                    on_wait=waits[:1], on_update=list(si.on_update)
                )
                for w in waits[1:]:
                    nop = self.nc.sync.nop(nofuse=True)
                    nop.ins.sync_info = mybir.SyncInfo(on_wait=[w], on_update=[])
            self.nc.sync.drain()
            self.nc.all_engine_barrier()
            popped = self.nc._tile_sem_poison_stack.pop()
            assert popped is self._sem_poison
            self.nc.clear_and_free_semaphores(list(self.sems.allocated().values()))
            self.nc.all_engine_barrier()

    return PatchedTileContext


def _build_program_spans(repeat: int = 1, span=2048, nchunk=NCHUNK,
                         xbufs=4, ybufs=4, psum_bufs=2):
    """Span-pipelined variant: the load/cast/matmul/merge/store pipeline unit
    is one `span`-column slice of a batch row (with K-1 halo columns), not a
    whole batch row."""
    import concourse.bass as bass
    import concourse.mybir as mybir

    TileContext = _make_tile_context_cls()

    f32 = mybir.dt.float32
    bf16 = mybir.dt.bfloat16
    spl = span + K - 1          # span + halo
    nspan = L // span
    mm_per_span = span // nchunk

    nc = bass.Bass()
    x_d = nc.declare_dram_parameter("x", [B, CPC, LP], f32, isOutput=False)
    wd_d = nc.declare_dram_parameter("wd", [CPC, (K - 1) * CPC], bf16, isOutput=False)
    w6_d = nc.declare_dram_parameter("w6", [CPC, 1], f32, isOutput=False)
    y_d = nc.declare_dram_parameter("y", [B, CPC, L], f32, isOutput=True)

    with TileContext(nc) as tc:
        with (
            tc.tile_pool(name="w", bufs=1) as wpool,
            tc.tile_pool(name="xb", bufs=xbufs) as xpool,
            tc.tile_pool(name="xh", bufs=xbufs) as xhpool,
            tc.tile_pool(name="yb", bufs=ybufs) as ypool,
            tc.tile_pool(name="ps", bufs=psum_bufs, space="PSUM") as pspool,
        ):
            wtile = wpool.tile([CPC, (K - 1) * CPC], bf16)
            nc.sync.dma_start(out=wtile[:], in_=wd_d[:])
            w6tile = wpool.tile([CPC, 1], f32)
            nc.sync.dma_start(out=w6tile[:], in_=w6_d[:])

            units = [(b, sp) for _ in range(repeat)
                     for b in range(B) for sp in range(nspan)]
            for b, sp in units:
                s0 = sp * span
                xt = xpool.tile([CPC, spl], f32)
                nc.sync.dma_start(out=xt[:], in_=x_d[b, :, s0 : s0 + spl])
                xh = xhpool.tile([CPC, spl], bf16)
                nc.scalar.activation(
                    xh[:], xt[:],
                    mybir.ActivationFunctionType.Copy,
                    bias=0.0, scale=1.0,
                )
                ps = pspool.tile([CPC, span], f32)
                for jj in range(mm_per_span):
                    for k in range(K - 1):
                        nc.tensor.matmul(
                            ps[:, jj * nchunk : (jj + 1) * nchunk],
                            lhsT=wtile[:, k * CPC : (k + 1) * CPC],
                            rhs=xh[:, jj * nchunk + k : jj * nchunk + k + nchunk],
                            start=(k == 0),
                            stop=(k == K - 2),
                        )
                yt = ypool.tile([CPC, span], f32)
                nc.vector.scalar_tensor_tensor(
                    out=yt[:],
                    in0=xt[:, K - 1 : spl],
                    scalar=w6tile[:, 0:1],
                    in1=ps[:],
                    op0=mybir.AluOpType.mult,
                    op1=mybir.AluOpType.add,
                )
                nc.sync.dma_start(out=y_d[b, :, s0 : s0 + span], in_=yt[:])
    return nc


def _build_program(repeat: int = 1, do_pe=True, do_act=True, do_cast=True,
                   do_add=True, nchunk=NCHUNK, fuse=False, psum_span=None,
                   dma_split=1, tap_outer=False, xbufs=3, ybufs=3):
    import concourse.bass as bass
    import concourse.mybir as mybir

    TileContext = _make_tile_context_cls()

    f32 = mybir.dt.float32
    bf16 = mybir.dt.bfloat16
    nch = L // nchunk
    if psum_span is None:
        psum_span = nchunk
    assert psum_span % nchunk == 0
    nspan = L // psum_span
    mm_per_span = psum_span // nchunk
    span_banks = (psum_span * 4 + 2047) // 2048
    psum_bufs = max(2, 8 // span_banks)

    nc = bass.Bass()
    x_d = nc.declare_dram_parameter("x", [B, CPC, LP], f32, isOutput=False)
    wd_d = nc.declare_dram_parameter("wd", [CPC, (K - 1) * CPC], bf16, isOutput=False)
    w6_d = nc.declare_dram_parameter("w6", [CPC, 1], f32, isOutput=False)
    y_d = nc.declare_dram_parameter("y", [B, CPC, L], f32, isOutput=True)

    with TileContext(nc) as tc:
        with (
            tc.tile_pool(name="w", bufs=1) as wpool,
            tc.tile_pool(name="xb", bufs=xbufs) as xpool,
            tc.tile_pool(name="xh", bufs=xbufs) as xhpool,
            tc.tile_pool(name="yb", bufs=ybufs) as ypool,
            tc.tile_pool(name="ps", bufs=psum_bufs, space="PSUM") as pspool,
        ):
            wtile = wpool.tile([CPC, (K - 1) * CPC], bf16)
            nc.sync.dma_start(out=wtile[:], in_=wd_d[:])
            w6tile = wpool.tile([CPC, 1], f32)
            nc.sync.dma_start(out=w6tile[:], in_=w6_d[:])

            for b in [b for _ in range(repeat) for b in range(B)]:
                xt = xpool.tile([CPC, LP], f32)
                if dma_split == 1:
                    nc.sync.dma_start(out=xt[:], in_=x_d[b])
                else:
                    step = LP // dma_split
                    for s in range(dma_split):
                        lo = s * step
                        hi = LP if s == dma_split - 1 else (s + 1) * step
                        nc.sync.dma_start(
                            out=xt[:, lo:hi], in_=x_d[b, :, lo:hi]
                        )
                need_yt = do_act or (do_pe and do_add)
                # bf16 copy of the padded row for the TensorE correction taps
                if do_cast:
                    xh = xhpool.tile([CPC, LP], bf16)
                    if fuse == "half":
                        # split the cast: ACT does the front half, DVE the
                        # back half, halving the latency gating the matmuls
                        mid = LP // 2
                        nc.scalar.activation(
                            xh[:, :mid], xt[:, :mid],
                            mybir.ActivationFunctionType.Copy,
                            bias=0.0, scale=1.0,
                        )
                        nc.vector.tensor_copy(out=xh[:, mid:], in_=xt[:, mid:])
                    elif fuse:
                        # ACT does the cast; DVE does the fused mul-add below
                        nc.scalar.activation(
                            xh[:], xt[:],
                            mybir.ActivationFunctionType.Copy,
                            bias=0.0, scale=1.0,
                        )
                    else:
                        nc.vector.tensor_copy(out=xh[:], in_=xt[:])
                yt = None
                if need_yt:
                    yt = ypool.tile([CPC, L], f32, tag="yt")
                # main tap: yt[c, l] = x[c, l] * w[c, K-1]
                if do_act and not fuse:
                    nc.scalar.activation(
                        yt[:],
                        xt[:, K - 1 : LP],
                        mybir.ActivationFunctionType.Copy,
                        bias=0.0,
                        scale=w6tile[:, 0:1],
                    )
                for sp in range(nspan):
                    if do_pe:
                        ps = pspool.tile([CPC, psum_span], f32)
                        if tap_outer:
                            mm_iter = [(jj, k) for k in range(K - 1)
                                       for jj in range(mm_per_span)]
                        else:
                            mm_iter = [(jj, k) for jj in range(mm_per_span)
                                       for k in range(K - 1)]
                        for jj, k in mm_iter:
                            j = sp * mm_per_span + jj
                            nc.tensor.matmul(
                                ps[:, jj * nchunk : (jj + 1) * nchunk],
                                lhsT=wtile[:, k * CPC : (k + 1) * CPC],
                                rhs=xh[:, j * nchunk + k : j * nchunk + k + nchunk],
                                start=(k == 0),
                                stop=(k == K - 2),
                            )
                    if do_pe and do_add:
                        s0 = sp * psum_span
                        if fuse:
                            # yt = x * w6 + corrections, one DVE op per span
                            nc.vector.scalar_tensor_tensor(
                                out=yt[:, s0 : s0 + psum_span],
                                in0=xt[:, s0 + K - 1 : s0 + psum_span + K - 1],
                                scalar=w6tile[:, 0:1],
                                in1=ps[:],
                                op0=mybir.AluOpType.mult,
                                op1=mybir.AluOpType.add,
                            )
                        elif do_act:
                            nc.vector.tensor_add(
                                out=yt[:, s0 : s0 + psum_span],
                                in0=yt[:, s0 : s0 + psum_span],
                                in1=ps[:],
                            )
                        else:
                            nc.vector.tensor_copy(
                                out=yt[:, s0 : s0 + psum_span],
                                in_=ps[:],
                            )
                src = yt[:] if need_yt else xt[:, K - 1 : LP]
                if dma_split == 1:
                    nc.sync.dma_start(out=y_d[b], in_=src)
                else:
                    step = L // dma_split
                    for s in range(dma_split):
                        lo = s * step
                        hi = L if s == dma_split - 1 else (s + 1) * step
                        nc.sync.dma_start(out=y_d[b, :, lo:hi], in_=src[:, lo:hi])
    return nc


def _pack_inputs(x: np.ndarray, filters: np.ndarray):
    import ml_dtypes

    xr = np.asarray(x, dtype=np.float32).reshape(B, L, C)
    w = np.asarray(filters, dtype=np.float32).reshape(C, K)

    xpad = np.zeros((B, C, LP), dtype=np.float32)
    xpad[:, :, K - 1 :] = xr.transpose(0, 2, 1)

    in_maps = []
    for i in range(NCORES):
        c0 = i * CPC
        xs = np.ascontiguousarray(xpad[:, c0 : c0 + CPC, :])
        wc = w[c0 : c0 + CPC]
        wd = np.zeros((CPC, (K - 1) * CPC), dtype=ml_dtypes.bfloat16)
        idx = np.arange(CPC)
        for k in range(K - 1):
            wd[idx, k * CPC + idx] = wc[:, k].astype(ml_dtypes.bfloat16)
        w6 = np.ascontiguousarray(wc[:, K - 1 : K])
        in_maps.append({"x": xs, "wd": wd, "w6": w6})
    return in_maps


def _unpack_outputs(results):
    y = np.empty((B, C, L), dtype=np.float32)
    for i in range(NCORES):
        y[:, i * CPC : (i + 1) * CPC, :] = results[i]["y"]
    return np.ascontiguousarray(y.transpose(0, 2, 1)).reshape(B, L, H, D)


# tuned configuration: fused DVE mul-add merge over 2048-col PSUM spans,
# ACT/DVE split bf16 cast, loads/stores split in two for queue parallelism
BEST_CFG = dict(fuse="half", psum_span=2048, dma_split=2)


def _run(x: np.ndarray, filters: np.ndarray, trace: bool = False):
    from concourse.bass_utils import run_bass_kernel_spmd

    nc = _build_program(**BEST_CFG)
    in_maps = _pack_inputs(x, filters)
    res = run_bass_kernel_spmd(nc, in_maps, list(range(NCORES)), trace=trace)
    return _unpack_outputs(res.results), res


def kernel(x: np.ndarray, filters: np.ndarray) -> np.ndarray:
    out, _ = _run(x, filters, trace=False)
    return out
